# revision 17
# baseline (speedup 1.0000x reference)
# Trainium2 Bass kernel for nn_LogicityVisReasoningEngine.
# Strategy: the reference returns only batch-0 outputs, and arity-3 NLM tensors
# are live only in layers 1-2. All heavy tensors are kept channel-major
# [C, positions] in SBUF; f3 uses an interleaved layout p = 16c + (i%16).
# The 6-permutation arity-3 matmul collapses to 3 broadcast planes (layer 1)
# and a 24-row staged matmul + swapped-rhs twin (layer 2).
import os
import numpy as np

N = 48
NN = N * N
FS = 3 * NN          # f31 free size per partition (3 ig blocks)
N3 = N * N * N
PIECES = {
    "W0p1": [8, 8], "W1p1": [8, 5, 5], "W2p1": [8, 5, 8, 5],
    "W0p2": [8, 8, 8], "W1p2": [8, 8, 8, 8], "W2p2": [8, 8, 16, 8, 8, 16],
    "W0p3": [8, 8, 8], "W1p3": [8, 8, 8, 8], "W2p3": [8, 8, 16, 8, 8, 16],
    "W1p4": [8, 8, 8, 8],
}
JCH = [(0, 10), (10, 10), (20, 10), (30, 10), (40, 8)]   # j-row chunks (<=512 free)

_compiled = None

def _trace():
    import concourse.bass as bass
    import concourse.mybir as mybir
    from concourse import bacc
    from concourse.tile import TileContext

    f32 = mybir.dt.float32
    AF = mybir.ActivationFunctionType
    ALU = mybir.AluOpType

    nc = bacc.Bacc("TRN2", target_bir_lowering=False, debug=False, num_devices=8)

    def din(name, shape):
        return nc.dram_tensor(name, shape, f32, kind="ExternalInput")

    # ---- inputs ----
    roiT = din("roiT", [128, 16 * 48])
    Wn1 = din("Wn1", [128, 16 * 512]); bn1 = din("bn1", [128, 4])
    Wn2 = din("Wn2", [128, 4 * 256]);  bn2 = din("bn2", [128, 2])
    Wn3 = din("Wn3", [128, 2 * 8]);    bn3 = din("bn3", [8, 1])
    X16 = din("X16", [16, NN])
    We1 = din("We1", [16, 256]); be1 = din("be1", [128, 2])
    We2 = din("We2", [128, 2 * 64]); be2 = din("be2", [64, 1])
    We3 = din("We3", [64, 4]); be3 = din("be3", [4, 1])
    priRow = din("priRow", [1, NN])
    # NLM weights (host pre-permuted)
    b01 = din("b01", [8, 1]); b11 = din("b11", [8, 1]); b21 = din("b21", [8, 1])
    A0 = din("A0", [5, 8]); A1 = din("A1", [5, 8]); A2 = din("A2", [5, 8]); A3 = din("A3", [5, 8])
    A4r = din("A4r", [5, 128]); A5r = din("A5r", [5, 128]); b3rep1 = din("b3rep1", [128, 1])
    b02 = din("b02", [8, 1]); b12 = din("b12", [8, 1]); b22 = din("b22", [8, 1])
    Ap0 = din("Ap0", [8, 8]); Ap1 = din("Ap1", [8, 8]); Ap2 = din("Ap2", [8, 8]); Ap3 = din("Ap3", [8, 8])
    Ap4r = din("Ap4r", [8, 128]); Ap5r = din("Ap5r", [8, 128]); b3rep2 = din("b3rep2", [128, 1])
    Wbig1 = din("Wbig1", [24, 143]); Wbig2 = din("Wbig2", [24, 143])
    b03 = din("b03", [8, 1]); b13 = din("b13", [8, 1]); b23 = din("b23", [8, 1]); b14 = din("b14", [8, 1])
    predW = din("predW", [8, 4]); predb = din("predb", [4, 1])
    piece_t = {}
    for pnm, sizes in PIECES.items():
        for qi, sz in enumerate(sizes):
            piece_t[f"{pnm}_{qi}"] = din(f"{pnm}_{qi}", [sz, 8])

    ncT_o = nc.dram_tensor("ncT_o", [8, 48], f32, kind="ExternalOutput")
    e5T_o = nc.dram_tensor("e5T_o", [5, NN], f32, kind="ExternalOutput")
    pred_o = nc.dram_tensor("pred_o", [4, 48], f32, kind="ExternalOutput")

    f31d = nc.dram_tensor("f31d", [8, N3], f32)   # internal: f3_1 c-major
    kcd = nc.dram_tensor("kcd", [8, N3], f32)     # internal: f3_1 (j<->k swapped)

    def sw(xT, C):
        # swapped (a,b)->(b,a) full view of c-major [C, NN]
        return xT[:].rearrange("p (a b) -> p a b", a=N).transpose([0, 2, 1])

    def swchunk(xT, j0, cnt):
        return xT[:].rearrange("p (a b) -> p a b", a=N).transpose([0, 2, 1])[:, j0:j0 + cnt, :]

    with TileContext(nc) as tc:
      from contextlib import ExitStack
      with ExitStack() as ctx:
        cpool = ctx.enter_context(tc.tile_pool(name="consts", bufs=1))
        work = ctx.enter_context(tc.tile_pool(name="work", bufs=3))
        big = ctx.enter_context(tc.tile_pool(name="big", bufs=1))
        from contextlib import ExitStack as _ES
        ppbox = {}
        pp_stack = _ES()
        class _PP:
            def tile(self, shape, dt, tag=None):
                return ppbox["pp"].tile(shape, dt, tag=tag, name=tag or "ps")
        pp = _PP()

        def load(t, shape, nm):
            s = cpool.tile(shape, f32, tag=nm, name=nm)
            nc.sync.dma_start(s[:], t[:])
            return s

        # persistent small tensors
        sb = {}
        X16s = cpool.tile([16, NN], f32, tag="r3", name="X16s")
        nc.sync.dma_start(X16s[:], X16[:])
        sb["X16"] = X16s
        for nm, t, shp in [
            ("We1", We1, [16, 256]), ("be1", be1, [128, 2]),
            ("We2", We2, [128, 128]), ("be2", be2, [64, 1]), ("We3", We3, [64, 4]), ("be3", be3, [4, 1]),
            ("b01", b01, [8, 1]), ("b11", b11, [8, 1]),
            ("b21", b21, [8, 1]),
            ("A0", A0, [5, 8]), ("A1", A1, [5, 8]), ("A2", A2, [5, 8]), ("A3", A3, [5, 8]),
            ("A4r", A4r, [5, 128]), ("A5r", A5r, [5, 128]), ("b3rep1", b3rep1, [128, 1]),
            ("b02", b02, [8, 1]), ("b12", b12, [8, 1]),
            ("b22", b22, [8, 1]),
            ("Ap0", Ap0, [8, 8]), ("Ap1", Ap1, [8, 8]), ("Ap2", Ap2, [8, 8]), ("Ap3", Ap3, [8, 8]),
            ("Ap4r", Ap4r, [8, 128]), ("Ap5r", Ap5r, [8, 128]), ("b3rep2", b3rep2, [128, 1]),
            ("Wbig1", Wbig1, [24, 143]), ("Wbig2", Wbig2, [24, 143]),
            ("b03", b03, [8, 1]), ("b13", b13, [8, 1]),
            ("b23", b23, [8, 1]), ("b14", b14, [8, 1]),
            ("predW", predW, [8, 4]), ("predb", predb, [4, 1]),
            ("bn3", bn3, [8, 1]),
        ]:
            sb[nm] = load(t, shp, nm)
        for pnm, sizes in PIECES.items():
            for qi, sz in enumerate(sizes):
                sb[f"{pnm}_{qi}"] = load(piece_t[f"{pnm}_{qi}"], [sz, 8], f"{pnm}_{qi}")

        # ---------------- node MLP ----------------
        with tc.tile_pool(name="node", bufs=1) as npool, \
             tc.tile_pool(name="ppn", bufs=2, space="PSUM") as ppn:
            roiS = npool.tile([128, 16 * 48], f32); nc.sync.dma_start(roiS[:], roiT[:])
            bn1S = npool.tile([128, 4], f32); nc.sync.dma_start(bn1S[:], bn1[:])
            Wn2S = npool.tile([128, 4 * 256], f32); nc.sync.dma_start(Wn2S[:], Wn2[:])
            bn2S = npool.tile([128, 2], f32); nc.sync.dma_start(bn2S[:], bn2[:])
            Wn3S = npool.tile([128, 2 * 8], f32); nc.sync.dma_start(Wn3S[:], Wn3[:])
            h1 = npool.tile([128, 4 * 48], f32)
            Wn1v = Wn1[:].rearrange("p (k m) -> p k m", k=16)
            for m in range(4):
                W1m = work.tile([128, 16 * 128], f32, tag="wk9")
                nc.sync.dma_start(W1m[:], Wn1v[:, :, m * 128:(m + 1) * 128])
                W1mv = W1m[:].rearrange("p (k m) -> p k m", k=16)
                ps = ppn.tile([128, 48], f32, tag="nodeps")
                for k in range(16):
                    nc.tensor.matmul(ps[:], W1mv[:, k, :],
                                     roiS[:, k * 48:(k + 1) * 48],
                                     start=(k == 0), stop=(k == 15))
                nc.scalar.activation(h1[:, m * 48:(m + 1) * 48], ps[:], AF.Relu,
                                     bias=bn1S[:, m:m + 1])
            W2v = Wn2S[:].rearrange("p (k m) -> p k m", k=4)
            h2 = npool.tile([128, 2 * 48], f32)
            for mm in range(2):
                ps = ppn.tile([128, 48], f32, tag="nodeps")
                for m in range(4):
                    nc.tensor.matmul(ps[:], W2v[:, m, mm * 128:(mm + 1) * 128],
                                     h1[:, m * 48:(m + 1) * 48],
                                     start=(m == 0), stop=(m == 3))
                nc.scalar.activation(h2[:, mm * 48:(mm + 1) * 48], ps[:], AF.Relu,
                                     bias=bn2S[:, mm:mm + 1])
            W3v = Wn3S[:].rearrange("p (k m) -> p k m", k=2)
            ps = ppn.tile([8, 48], f32, tag="smallps")
            for k in range(2):
                nc.tensor.matmul(ps[:], W3v[:, k, :], h2[:, k * 48:(k + 1) * 48],
                                 start=(k == 0), stop=(k == 1))
            ncT = cpool.tile([8, 48], f32)
            nc.scalar.activation(ncT[:], ps[:], AF.Sigmoid, bias=sb["bn3"][:, 0:1])
            nc.sync.dma_start(ncT_o[:], ncT[:])

        # ---------------- edge MLP ----------------
        e5T = cpool.tile([5, NN], f32, tag="f2even", name="e5T")
        with tc.tile_pool(name="edge", bufs=1) as epool, \
             tc.tile_pool(name="ppe", bufs=2, space="PSUM") as ppe:
            ECH = [(0, 512), (512, 512), (1024, 512), (1536, 512), (2048, 256)]
            h1e = epool.tile([128, 2 * NN], f32)
            for mm in range(2):
                for f0, fc in ECH:
                    ps = ppe.tile([128, 512], f32, tag="edgeps")
                    nc.tensor.matmul(ps[:, :fc], sb["We1"][:, mm * 128:(mm + 1) * 128],
                                     sb["X16"][:, f0:f0 + fc], start=True, stop=True)
                    nc.scalar.activation(h1e[:, mm * NN + f0: mm * NN + f0 + fc],
                                         ps[:, :fc], AF.Relu, bias=sb["be1"][:, mm:mm + 1])
            h2e = epool.tile([64, NN], f32)
            for f0, fc in ECH:
                ps = ppe.tile([64, 512], f32, tag="edgeps2")
                for mm in range(2):
                    nc.tensor.matmul(ps[:, :fc], sb["We2"][:, mm * 64:(mm + 1) * 64],
                                     h1e[:, mm * NN + f0: mm * NN + f0 + fc],
                                     start=(mm == 0), stop=(mm == 1))
                nc.scalar.activation(h2e[:, f0:f0 + fc], ps[:, :fc], AF.Relu,
                                     bias=sb["be2"][:, 0:1])
            for f0, fc in ECH:
                ps = ppe.tile([4, 512], f32, tag="edgeps3")
                nc.tensor.matmul(ps[:, :fc], sb["We3"][:], h2e[:, f0:f0 + fc],
                                 start=True, stop=True)
                nc.scalar.activation(e5T[0:4, f0:f0 + fc], ps[:, :fc], AF.Sigmoid,
                                     bias=sb["be3"][:, 0:1])
        nc.sync.dma_start(e5T[4:5, :], priRow[:])
        diag = bass.AP(e5T.tensor, e5T[:].offset, [[NN, 5], [N + 1, N]])
        nc.vector.memset(diag, 0.0)
        nc.sync.dma_start(e5T_o[:], e5T[:])

        # ---------------- NLM small helpers ----------------
        ppbox["pp"] = pp_stack.enter_context(tc.tile_pool(name="pp1", bufs=1, space="PSUM"))
        def mm_sig(out, pieces, bias, psshape, tag):
            # pieces: list of (lhsT_ap, rhs_ap); accumulate then sigmoid+bias
            ps = pp.tile(psshape, f32, tag=tag)
            fs = out.free_size()
            psv = ps[:, :fs]
            nparts = len(pieces)
            for q, (l, r) in enumerate(pieces):
                nc.tensor.matmul(psv, l, r, start=(q == 0), stop=(q == nparts - 1))
            nc.scalar.activation(out, psv, AF.Sigmoid, bias=bias)

        # reduce helpers on c-major [C, NN]
        def masked_reduce(xT, C, Et, Ft):
            tmp = work.tile([C, NN], f32, tag="wk9")
            nc.vector.tensor_copy(tmp[:], xT[:])
            dg = bass.AP(tmp.tensor, tmp[:].offset, [[NN, C], [N + 1, N]])
            nc.vector.memset(dg, 0.0)
            nc.vector.tensor_reduce(Et, tmp[:].rearrange("p (a b) -> p a b", a=N),
                                    mybir.AxisListType.X, ALU.max)
            nc.vector.memset(dg, 1.0)
            nc.vector.tensor_reduce(Ft, tmp[:].rearrange("p (a b) -> p a b", a=N),
                                    mybir.AxisListType.X, ALU.min)

        # ---------------- NLM layer 1 (small parts) ----------------
        E1 = cpool.tile([8, 1], f32); F1 = cpool.tile([8, 1], f32)
        nc.vector.tensor_reduce(E1[:], ncT[:], mybir.AxisListType.X, ALU.max)
        nc.vector.tensor_reduce(F1[:], ncT[:], mybir.AxisListType.X, ALU.min)
        f0_1 = cpool.tile([8, 1], f32)
        mm_sig(f0_1[:], [(sb["W0p1_0"][:], E1[:]), (sb["W0p1_1"][:], F1[:])],
               sb["b01"][:, 0:1], [8, 1], "ps81")
        E2 = cpool.tile([5, 48], f32); F2 = cpool.tile([5, 48], f32)
        masked_reduce(e5T, 5, E2[:], F2[:])
        f1_1 = cpool.tile([8, 48], f32)
        mm_sig(f1_1[:], [(sb["W1p1_0"][:], ncT[:]), (sb["W1p1_1"][:], E2[:]),
                         (sb["W1p1_2"][:], F2[:])], sb["b11"][:, 0:1], [8, 48], "ps848")
        # f2_1
        ncb = cpool.tile([8, NN], f32, tag="shA")
        nc.vector.tensor_copy(ncb[:].rearrange("p (a b) -> p a b", a=N),
                              ncT[:].unsqueeze(2).broadcast_to((8, N, N)))
        ncb2 = cpool.tile([8, NN], f32, tag="shB")
        nc.vector.tensor_copy(ncb2[:].rearrange("p (a b) -> p a b", a=N),
                              ncT[:].unsqueeze(1).broadcast_to((8, N, N)))
        f2_1 = cpool.tile([8, NN], f32, tag="f2odd")
        for j0, cnt in JCH:
            s = slice(j0 * 48, (j0 + cnt) * 48)
            mm_sig(f2_1[:, s],
                   [(sb["W2p1_0"][:], ncb[:, s]), (sb["W2p1_1"][:], e5T[:, s]),
                    (sb["W2p1_2"][:], ncb2[:, s]), (sb["W2p1_3"][:], swchunk(e5T, j0, cnt))],
                   sb["b21"][:, 0:1], [8, 480], "ps8480")

        # ---------------- GG planes for f3_1 ----------------
        GG0s = cpool.tile([8, NN], f32, tag="shA"); GG1s = cpool.tile([8, NN], f32, tag="shB")
        JCH8 = [(a0, 8) for a0 in range(0, 48, 8)]
        def gg_dst_ap(dst, a0):
            ig0, im0 = a0 // 16, a0 % 16
            return bass.AP(dst.tensor, dst[:].offset + im0 * 144 + ig0 * 48,
                           [[NN, 8], [144, 8], [1, 48]])
        for (dst, Ad, At) in [(GG0s, "A0", "A2"), (GG1s, "A1", "A3")]:
            for j0, cnt in JCH8:
                s = slice(j0 * 48, (j0 + cnt) * 48)
                ps = pp.tile([8, 480], f32, tag="ggps")
                nc.tensor.matmul(ps[:, :cnt * 48], sb[Ad][:], e5T[:, s], start=True, stop=False)
                nc.tensor.matmul(ps[:, :cnt * 48], sb[At][:], swchunk(e5T, j0, cnt),
                                 start=False, stop=True)
                nc.vector.tensor_copy(gg_dst_ap(dst, j0),
                                      ps[:, :cnt * 48].rearrange("p (a x) -> p a x", a=8))
        R2a = big.tile([128, NN], f32, tag="r2")
        for j0, cnt in JCH:
            s = slice(j0 * 48, (j0 + cnt) * 48)
            ps = pp.tile([128, 480], f32, tag="ggrep")
            nc.tensor.matmul(ps[:, :cnt * 48], sb["A4r"][:], e5T[:, s], start=True, stop=False)
            nc.tensor.matmul(ps[:, :cnt * 48], sb["A5r"][:], swchunk(e5T, j0, cnt),
                             start=False, stop=True)
            nc.vector.tensor_copy(R2a[:, s], ps[:, :cnt * 48])
        R01a = big.tile([128, 288], f32, tag="r01")
        for c in range(8):
            for t, s2 in [(0, GG0s), (1, GG1s)]:
                sap = bass.AP(s2.tensor, s2[:].offset + c * NN, [[NN, 1], [1, NN]])
                dap = bass.AP(R01a.tensor, R01a[:].offset + 16 * c * 288 + t * 144,
                              [[288, 16], [1, 144]])
                nc.sync.dma_start(dap, sap)

        # ---------------- f3_1 build + exports + reduce ----------------
        f31 = big.tile([128, FS], f32)
        R01v = R01a[:].rearrange("p (t g x) -> p t g x", t=2, g=3)
        EL = big.tile([128, 144], f32, tag="el"); FL = big.tile([128, 144], f32, tag="fl")
        for ig in range(3):
            tb = work.tile([128, NN], f32, tag="wk9")
            nc.vector.tensor_add(tb[:].rearrange("p (a b) -> p a b", a=N),
                                 R01v[:, 0, ig, :].unsqueeze(2).broadcast_to((128, N, N)),
                                 R01v[:, 1, ig, :].unsqueeze(1).broadcast_to((128, N, N)))
            tb2 = work.tile([128, NN], f32, tag="wk9")
            nc.vector.tensor_add(tb2[:], tb[:], R2a[:])
            nc.scalar.activation(f31[:, ig * NN:(ig + 1) * NN], tb2[:], AF.Sigmoid,
                                 bias=sb["b3rep1"][:, 0:1])
            # exports: f31d per (c, ig)
            for c in range(8):
                sap = bass.AP(f31.tensor, f31[:].offset + 16 * c * FS + ig * NN, [[FS, 16], [1, NN]])
                dap = bass.AP(f31d, c * N3 + 16 * ig * NN, [[NN, 16], [1, NN]])
                nc.sync.dma_start(dap, sap)
            # kc slice (j<->k swap) + export
            kcs = work.tile([128, NN], f32, tag="wk9")
            nc.vector.tensor_copy(kcs[:].rearrange("p (a b) -> p a b", a=N),
                                  f31[:, ig * NN:(ig + 1) * NN].rearrange("p (a b) -> p a b", a=N).transpose([0, 2, 1]))
            for c in range(8):
                sap = bass.AP(kcs.tensor, kcs[:].offset + 16 * c * NN, [[NN, 16], [1, NN]])
                dap = bass.AP(kcd, c * N3 + 16 * ig * NN, [[NN, 16], [1, NN]])
                nc.sync.dma_start(dap, sap)
            # masked reduce over k
            tmp = work.tile([128, NN], f32, tag="wk9")
            nc.vector.tensor_copy(tmp[:], f31[:, ig * NN:(ig + 1) * NN])
            dg = bass.AP(tmp.tensor, tmp[:].offset, [[NN, 128], [N + 1, N]])
            nc.vector.memset(dg, 0.0)
            nc.vector.tensor_reduce(EL[:, ig * 48:(ig + 1) * 48],
                                    tmp[:].rearrange("p (a b) -> p a b", a=N),
                                    mybir.AxisListType.X, ALU.max)
            nc.vector.memset(dg, 1.0)
            nc.vector.tensor_reduce(FL[:, ig * 48:(ig + 1) * 48],
                                    tmp[:].rearrange("p (a b) -> p a b", a=N),
                                    mybir.AxisListType.X, ALU.min)
        r3_1 = cpool.tile([16, NN], f32, tag="r3")
        for c in range(8):
            for half, s3 in [(0, EL), (1, FL)]:
                for g in range(3):
                    sap = bass.AP(s3.tensor, s3[:].offset + c * 16 * 144 + g * 48, [[144, 16], [1, 48]])
                    dap = bass.AP(r3_1.tensor, r3_1[:].offset + (half * 8 + c) * NN + g * 768,
                                  [[NN, 1], [48, 16], [1, 48]])
                    nc.sync.dma_start(dap, sap)

        # ---------------- layer 2 smalls ----------------
        E11 = cpool.tile([8, 1], f32); F11 = cpool.tile([8, 1], f32)
        nc.vector.tensor_reduce(E11[:], f1_1[:], mybir.AxisListType.X, ALU.max)
        nc.vector.tensor_reduce(F11[:], f1_1[:], mybir.AxisListType.X, ALU.min)
        f0_2 = cpool.tile([8, 1], f32)
        mm_sig(f0_2[:], [(sb["W0p2_0"][:], f0_1[:]), (sb["W0p2_1"][:], E11[:]),
                         (sb["W0p2_2"][:], F11[:])], sb["b02"][:, 0:1], [8, 1], "ps81")
        E21 = cpool.tile([8, 48], f32); F21 = cpool.tile([8, 48], f32)
        masked_reduce(f2_1, 8, E21[:], F21[:])
        f01b = cpool.tile([8, 48], f32)
        nc.vector.tensor_copy(f01b[:], f0_1[:].broadcast_to((8, 48)))
        f1_2 = cpool.tile([8, 48], f32)
        mm_sig(f1_2[:], [(sb["W1p2_0"][:], f01b[:]), (sb["W1p2_1"][:], f1_1[:]),
                         (sb["W1p2_2"][:], E21[:]), (sb["W1p2_3"][:], F21[:])],
               sb["b12"][:, 0:1], [8, 48], "ps848")
        f11b = cpool.tile([8, NN], f32, tag="shA")
        nc.vector.tensor_copy(f11b[:].rearrange("p (a b) -> p a b", a=N),
                              f1_1[:].unsqueeze(2).broadcast_to((8, N, N)))
        f11b2 = cpool.tile([8, NN], f32, tag="shB")
        nc.vector.tensor_copy(f11b2[:].rearrange("p (a b) -> p a b", a=N),
                              f1_1[:].unsqueeze(1).broadcast_to((8, N, N)))
        f2_2 = cpool.tile([8, NN], f32, tag="f2even")
        for j0, cnt in JCH:
            s = slice(j0 * 48, (j0 + cnt) * 48)
            mm_sig(f2_2[:, s],
                   [(sb["W2p2_0"][:], f11b[:, s]), (sb["W2p2_1"][:], f2_1[:, s]),
                    (sb["W2p2_2"][:], r3_1[:, s]), (sb["W2p2_3"][:], f11b2[:, s]),
                    (sb["W2p2_4"][:], swchunk(f2_1, j0, cnt)),
                    (sb["W2p2_5"][:], swchunk(r3_1, j0, cnt))],
                   sb["b22"][:, 0:1], [8, 480], "ps8480")

        # ---------------- base3 for f3_2 ----------------
        GG0p = cpool.tile([8, NN], f32, tag="shA"); GG1p = cpool.tile([8, NN], f32, tag="shB")
        for (dst, Ad, At) in [(GG0p, "Ap0", "Ap2"), (GG1p, "Ap1", "Ap3")]:
            for j0, cnt in JCH8:
                s = slice(j0 * 48, (j0 + cnt) * 48)
                ps = pp.tile([8, 480], f32, tag="ggps")
                nc.tensor.matmul(ps[:, :cnt * 48], sb[Ad][:], f2_1[:, s], start=True, stop=False)
                nc.tensor.matmul(ps[:, :cnt * 48], sb[At][:], swchunk(f2_1, j0, cnt),
                                 start=False, stop=True)
                nc.vector.tensor_copy(gg_dst_ap(dst, j0),
                                      ps[:, :cnt * 48].rearrange("p (a x) -> p a x", a=8))
        R2b = big.tile([128, NN], f32, tag="r2")
        for j0, cnt in JCH:
            s = slice(j0 * 48, (j0 + cnt) * 48)
            ps = pp.tile([128, 480], f32, tag="ggrep")
            nc.tensor.matmul(ps[:, :cnt * 48], sb["Ap4r"][:], f2_1[:, s], start=True, stop=False)
            nc.tensor.matmul(ps[:, :cnt * 48], sb["Ap5r"][:], swchunk(f2_1, j0, cnt),
                             start=False, stop=True)
            nc.vector.tensor_copy(R2b[:, s], ps[:, :cnt * 48])
        R01b = big.tile([128, 288], f32, tag="r01")
        for c in range(8):
            for t, s2 in [(0, GG0p), (1, GG1p)]:
                sap = bass.AP(s2.tensor, s2[:].offset + c * NN, [[NN, 1], [1, NN]])
                dap = bass.AP(R01b.tensor, R01b[:].offset + 16 * c * 288 + t * 144,
                              [[288, 16], [1, 144]])
                nc.sync.dma_start(dap, sap)
        base3 = big.tile([128, FS], f32)
        R01bv = R01b[:].rearrange("p (t g x) -> p t g x", t=2, g=3)
        for ig in range(3):
            tb = work.tile([128, NN], f32, tag="wk9")
            nc.vector.tensor_add(tb[:].rearrange("p (a b) -> p a b", a=N),
                                 R01bv[:, 0, ig, :].unsqueeze(2).broadcast_to((128, N, N)),
                                 R01bv[:, 1, ig, :].unsqueeze(1).broadcast_to((128, N, N)))
            nc.vector.tensor_add(base3[:, ig * NN:(ig + 1) * NN], tb[:], R2b[:])

        # ---------------- f3_2: staged perm matmuls ----------------
        pp_stack.close()
        ppf3_stack = _ES()
        ppf3 = ppf3_stack.enter_context(tc.tile_pool(name="ppf3", bufs=1, space="PSUM"))
        EL2 = big.tile([128, 144], f32, tag="el"); FL2 = big.tile([128, 144], f32, tag="fl")
        stg = ctx.enter_context(tc.tile_pool(name="stg", bufs=2))
        for ig in range(3):
            pss = []
            for q, (j0, cnt) in enumerate(JCH):
                pst = ppf3.tile([128, cnt * 48], f32, tag=f"f3ps{q}")
                pss.append(pst)
            for im in range(16):
                i = 16 * ig + im
                Xi = stg.tile([24, NN], f32, tag="Xi")
                # m2 block (rows 0-7): f31[j,i,k] from f31d
                sap = bass.AP(f31d, i * 48, [[N3, 8], [NN, 48], [1, 48]])
                dap = bass.AP(Xi.tensor, Xi[:].offset, [[NN, 8], [48, 48], [1, 48]])
                nc.sync.dma_start(dap, sap)
                # m4 block (rows 8-15): f31[j,k,i] from kcd
                sap = bass.AP(kcd, i * 48, [[N3, 8], [NN, 48], [1, 48]])
                dap = bass.AP(Xi.tensor, Xi[:].offset + 8 * NN, [[NN, 8], [48, 48], [1, 48]])
                nc.sync.dma_start(dap, sap)
                # m0 block (rows 16-23): f31[i,j,k] from SBUF
                sap = bass.AP(f31.tensor, f31[:].offset + im * FS + ig * NN, [[16 * FS, 8], [1, NN]])
                dap = bass.AP(Xi.tensor, Xi[:].offset + 16 * NN, [[NN, 8], [1, NN]])
                nc.sync.dma_start(dap, sap)
                for q, (j0, cnt) in enumerate(JCH):
                    s = slice(j0 * 48, (j0 + cnt) * 48)
                    nc.tensor.matmul(pss[q][:], sb["Wbig1"][:, 15 - im:143 - im],
                                     Xi[:, s], start=(im == 0), stop=False)
                    nc.tensor.matmul(pss[q][:], sb["Wbig2"][:, 15 - im:143 - im],
                                     swchunk(Xi, j0, cnt), start=False, stop=(im == 15))
            f32s = work.tile([128, NN], f32, tag="wk9")
            for q, (j0, cnt) in enumerate(JCH):
                s = slice(j0 * 48, (j0 + cnt) * 48)
                tb = work.tile([128, 480], f32, tag="f32tb")
                nc.vector.tensor_add(tb[:, :cnt * 48], pss[q][:], base3[:, ig * NN + j0 * 48: ig * NN + (j0 + cnt) * 48])
                nc.scalar.activation(f32s[:, s], tb[:, :cnt * 48], AF.Sigmoid,
                                     bias=sb["b3rep2"][:, 0:1])
            tmp = f32s
            dg = bass.AP(tmp.tensor, tmp[:].offset, [[NN, 128], [N + 1, N]])
            nc.vector.memset(dg, 0.0)
            nc.vector.tensor_reduce(EL2[:, ig * 48:(ig + 1) * 48],
                                    tmp[:].rearrange("p (a b) -> p a b", a=N),
                                    mybir.AxisListType.X, ALU.max)
            nc.vector.memset(dg, 1.0)
            nc.vector.tensor_reduce(FL2[:, ig * 48:(ig + 1) * 48],
                                    tmp[:].rearrange("p (a b) -> p a b", a=N),
                                    mybir.AxisListType.X, ALU.min)
        r3_2 = cpool.tile([16, NN], f32, tag="r3")
        for c in range(8):
            for half, s3 in [(0, EL2), (1, FL2)]:
                for g in range(3):
                    sap = bass.AP(s3.tensor, s3[:].offset + c * 16 * 144 + g * 48, [[144, 16], [1, 48]])
                    dap = bass.AP(r3_2.tensor, r3_2[:].offset + (half * 8 + c) * NN + g * 768,
                                  [[NN, 1], [48, 16], [1, 48]])
                    nc.sync.dma_start(dap, sap)

        # ---------------- layers 3-4 + pred ----------------
        ppf3_stack.close()
        ppbox["pp"] = ctx.enter_context(tc.tile_pool(name="pp2", bufs=1, space="PSUM"))
        E12 = cpool.tile([8, 1], f32); F12 = cpool.tile([8, 1], f32)
        nc.vector.tensor_reduce(E12[:], f1_2[:], mybir.AxisListType.X, ALU.max)
        nc.vector.tensor_reduce(F12[:], f1_2[:], mybir.AxisListType.X, ALU.min)
        f0_3 = cpool.tile([8, 1], f32)
        mm_sig(f0_3[:], [(sb["W0p3_0"][:], f0_2[:]), (sb["W0p3_1"][:], E12[:]),
                         (sb["W0p3_2"][:], F12[:])], sb["b03"][:, 0:1], [8, 1], "ps81")
        E22 = cpool.tile([8, 48], f32); F22 = cpool.tile([8, 48], f32)
        masked_reduce(f2_2, 8, E22[:], F22[:])
        f02b = cpool.tile([8, 48], f32)
        nc.vector.tensor_copy(f02b[:], f0_2[:].broadcast_to((8, 48)))
        f1_3 = cpool.tile([8, 48], f32)
        mm_sig(f1_3[:], [(sb["W1p3_0"][:], f02b[:]), (sb["W1p3_1"][:], f1_2[:]),
                         (sb["W1p3_2"][:], E22[:]), (sb["W1p3_3"][:], F22[:])],
               sb["b13"][:, 0:1], [8, 48], "ps848")
        f12b = cpool.tile([8, NN], f32, tag="shA")
        nc.vector.tensor_copy(f12b[:].rearrange("p (a b) -> p a b", a=N),
                              f1_2[:].unsqueeze(2).broadcast_to((8, N, N)))
        f12b2 = cpool.tile([8, NN], f32, tag="shB")
        nc.vector.tensor_copy(f12b2[:].rearrange("p (a b) -> p a b", a=N),
                              f1_2[:].unsqueeze(1).broadcast_to((8, N, N)))
        f2_3 = cpool.tile([8, NN], f32, tag="f2odd")
        for j0, cnt in JCH:
            s = slice(j0 * 48, (j0 + cnt) * 48)
            mm_sig(f2_3[:, s],
                   [(sb["W2p3_0"][:], f12b[:, s]), (sb["W2p3_1"][:], f2_2[:, s]),
                    (sb["W2p3_2"][:], r3_2[:, s]), (sb["W2p3_3"][:], f12b2[:, s]),
                    (sb["W2p3_4"][:], swchunk(f2_2, j0, cnt)),
                    (sb["W2p3_5"][:], swchunk(r3_2, j0, cnt))],
                   sb["b23"][:, 0:1], [8, 480], "ps8480")
        E23 = cpool.tile([8, 48], f32); F23 = cpool.tile([8, 48], f32)
        masked_reduce(f2_3, 8, E23[:], F23[:])
        f03b = cpool.tile([8, 48], f32)
        nc.vector.tensor_copy(f03b[:], f0_3[:].broadcast_to((8, 48)))
        f1_4 = cpool.tile([8, 48], f32)
        mm_sig(f1_4[:], [(sb["W1p4_0"][:], f03b[:]), (sb["W1p4_1"][:], f1_3[:]),
                         (sb["W1p4_2"][:], E23[:]), (sb["W1p4_3"][:], F23[:])],
               sb["b14"][:, 0:1], [8, 48], "ps848")
        ps = pp.tile([4, 48], f32, tag="ps848")
        nc.tensor.matmul(ps[:], sb["predW"][:], f1_4[:], start=True, stop=True)
        predS = cpool.tile([4, 48], f32)
        nc.vector.tensor_scalar_add(predS[:], ps[:], sb["predb"][:, 0:1])
        nc.sync.dma_start(pred_o[:], predS[:])

    nc.compile()
    return nc


def _prep_inputs(roi_features, batch_bboxes, batch_directions, batch_priorities,
                 node_mlp, edge_mlp, nlm_params, pred_W, pred_b):
    f = np.float32
    roi = np.asarray(roi_features, f)[0]
    bbox = np.asarray(batch_bboxes, f)[0]
    dirs = np.asarray(batch_directions, f)[0]
    pv = np.asarray(batch_priorities, f)[0]
    nm = [np.ascontiguousarray(np.asarray(x, f)) for x in node_mlp]
    em = [np.ascontiguousarray(np.asarray(x, f)) for x in edge_mlp]
    nlm = [[(np.asarray(W, f), np.asarray(b, f)) for (W, b) in layer] for layer in nlm_params]
    pW = np.asarray(pred_W, f); pb = np.asarray(pred_b, f)

    d = {}
    d["roiT"] = np.ascontiguousarray(roi.T.reshape(16, 128, 48).transpose(1, 0, 2).reshape(128, 768))
    d["Wn1"] = np.ascontiguousarray(nm[0].reshape(16, 128, 512).transpose(1, 0, 2).reshape(128, 16 * 512))
    d["bn1"] = np.ascontiguousarray(nm[1].reshape(4, 128).T)
    d["Wn2"] = np.ascontiguousarray(nm[2].reshape(4, 128, 256).transpose(1, 0, 2).reshape(128, 1024))
    d["bn2"] = np.ascontiguousarray(nm[3].reshape(2, 128).T)
    d["Wn3"] = np.ascontiguousarray(nm[4].reshape(2, 128, 8).transpose(1, 0, 2).reshape(128, 16))
    d["bn3"] = nm[5].reshape(8, 1)
    attr = np.concatenate([bbox / 1024.0, dirs], -1).astype(f)
    X16 = np.empty((16, NN), f)
    for c in range(8):
        X16[c] = np.repeat(attr[:, c], N)
        X16[8 + c] = np.tile(attr[:, c], N)
    d["X16"] = X16
    d["We1"] = em[0]
    d["be1"] = np.ascontiguousarray(em[1].reshape(2, 128).T)
    d["We2"] = np.ascontiguousarray(em[2].reshape(2, 128, 64).transpose(1, 0, 2).reshape(128, 128))
    d["be2"] = em[3].reshape(64, 1)
    d["We3"] = em[4]
    d["be3"] = em[5].reshape(4, 1)
    d["priRow"] = (pv[:, None] > pv[None, :]).astype(f).reshape(1, NN)

    def permW_reduce(W, off, C):
        Wn = W.copy()
        inter = np.concatenate([2 * np.arange(C), 2 * np.arange(C) + 1])
        Wn[off:off + 2 * C] = W[off + inter]
        return Wn

    (W0, b0), (W1, b1), (W2, b2), (W3, b3) = nlm[0]
    d["W0p1"] = permW_reduce(W0, 0, 8); d["b01"] = b0.reshape(8, 1)
    d["W1p1"] = permW_reduce(W1, 8, 5); d["b11"] = b1.reshape(8, 1)
    d["W2p1"] = W2; d["b21"] = b2.reshape(8, 1)
    A = [W3[5 * m:5 * m + 5] for m in range(6)]
    d["A0"], d["A1"], d["A2"], d["A3"] = A[0], A[1], A[2], A[3]
    rep = np.zeros((5, 128), f); rep2 = np.zeros((5, 128), f)
    for c in range(8):
        rep[:, 16 * c:16 * c + 16] = A[4][:, c:c + 1]
        rep2[:, 16 * c:16 * c + 16] = A[5][:, c:c + 1]
    d["A4r"], d["A5r"] = rep, rep2
    b3r = np.zeros((128, 1), f)
    for c in range(8):
        b3r[16 * c:16 * c + 16, 0] = b3[c]
    d["b3rep1"] = b3r

    (W0, b0), (W1, b1), (W2, b2), (W3, b3) = nlm[1]
    d["W0p2"] = permW_reduce(W0, 8, 8); d["b02"] = b0.reshape(8, 1)
    d["W1p2"] = permW_reduce(W1, 16, 8); d["b12"] = b1.reshape(8, 1)
    W2p = permW_reduce(W2, 16, 8); W2p = permW_reduce(W2p, 48, 8)
    d["W2p2"] = W2p; d["b22"] = b2.reshape(8, 1)
    Ab = [W3[16 * m:16 * m + 8] for m in range(6)]
    Bb = [W3[16 * m + 8:16 * m + 16] for m in range(6)]
    d["Ap0"], d["Ap1"], d["Ap2"], d["Ap3"] = Ab[0], Ab[1], Ab[2], Ab[3]
    rep = np.zeros((8, 128), f); rep2 = np.zeros((8, 128), f)
    for c in range(8):
        rep[:, 16 * c:16 * c + 16] = Ab[4][:, c:c + 1]
        rep2[:, 16 * c:16 * c + 16] = Ab[5][:, c:c + 1]
    d["Ap4r"], d["Ap5r"] = rep, rep2
    b3r = np.zeros((128, 1), f)
    for c in range(8):
        b3r[16 * c:16 * c + 16, 0] = b3[c]
    d["b3rep2"] = b3r
    # Wbig1 rows [B2;B4;B0] direct, Wbig2 rows [B3;B5;B1] for swapped rhs
    Wst1 = np.concatenate([Bb[2], Bb[4], Bb[0]], 0)   # [24, 8]
    Wst2 = np.concatenate([Bb[3], Bb[5], Bb[1]], 0)
    for nm2, Wst in [("Wbig1", Wst1), ("Wbig2", Wst2)]:
        Ww = np.zeros((24, 143), f)
        for c in range(8):
            Ww[:, 16 * c + 15] = Wst[:, c]
        d[nm2] = Ww

    (W0, b0), (W1, b1), (W2, b2), _ = nlm[2]
    d["W0p3"] = permW_reduce(W0, 8, 8); d["b03"] = b0.reshape(8, 1)
    d["W1p3"] = permW_reduce(W1, 16, 8); d["b13"] = b1.reshape(8, 1)
    W2p = permW_reduce(W2, 16, 8); W2p = permW_reduce(W2p, 48, 8)
    d["W2p3"] = W2p; d["b23"] = b2.reshape(8, 1)
    _, (W1, b1), _, _ = nlm[3]
    d["W1p4"] = permW_reduce(W1, 16, 8); d["b14"] = b1.reshape(8, 1)
    d["predW"] = pW; d["predb"] = pb.reshape(4, 1)

    pieces_def = {
        "W0p1": [8, 8], "W1p1": [8, 5, 5], "W2p1": [8, 5, 8, 5],
        "W0p2": [8, 8, 8], "W1p2": [8, 8, 8, 8], "W2p2": [8, 8, 16, 8, 8, 16],
        "W0p3": [8, 8, 8], "W1p3": [8, 8, 8, 8], "W2p3": [8, 8, 16, 8, 8, 16],
        "W1p4": [8, 8, 8, 8],
    }
    for pnm, sizes in pieces_def.items():
        W = d.pop(pnm)
        off = 0
        for qi, sz in enumerate(sizes):
            d[f"{pnm}_{qi}"] = np.ascontiguousarray(W[off:off + sz])
            off += sz
    d = {k: np.ascontiguousarray(v, f) for k, v in d.items()}
    return d


def kernel(**inputs):
    global _compiled
    from concourse.bass_utils import run_bass_kernel_spmd
    if _compiled is None:
        _compiled = _trace()
    d = _prep_inputs(**inputs)
    res = run_bass_kernel_spmd(_compiled, [d] * 8, list(range(8)))
    r = res.results[0]
    ncpt = r["ncT_o"].T.copy()                       # [48, 8]
    pred = r["pred_o"].T.copy()                      # [48, 4]
    ef = r["e5T_o"].reshape(5, N, N).transpose(1, 2, 0)
    ii = np.arange(N)[:, None]; kk = np.arange(N - 1)[None, :]
    jj = kk + (kk >= ii)
    edge = ef[ii, jj].reshape(N * (N - 1), 5).copy()
    return pred, ncpt, edge
